# revision 8
# baseline (speedup 1.0000x reference)
"""Trainium2 Bass kernel for a pre-LN transformer block (B=16,N=1024,D=768,H=12).

Strategy: data-parallel over batch across 8 NeuronCores (2 sequences/core,
weights replicated, no collectives).

Per-core pipeline (all matmuls bf16 with fp32 PSUM accumulation; the 1e-5
layer-scale gammas make inner-block bf16 error invisible in the output):
  1. LN1 token-major via bn_stats/bn_aggr, affine on ScalarE, PE-transpose
     the normalized activations to d-major (xnT).
  2. QKV GEMM: q,k feature-major (heads land on partition pairs), v
     token-major augmented with a ones column per head.
  3. Attention per (head, batch): scores computed TRANSPOSED
     (S^T[k,q] = k·q) so softmax normalization is deferred; exp on ScalarE
     writes unnormalized P^T directly in the [k-part, q-free] layout the
     P@V matmul needs (no probability transposes anywhere). The ones
     column in v makes each P^T@v matmul also emit the softmax denominator
     as PSUM row 64; a reciprocal + GPSIMD partition-broadcast + multiply
     normalizes the 64 output rows.
  4. proj GEMM + residual (rank-1 ones-outer-product adds biases inside
     PSUM), LN2, MLP with exact-GELU on ScalarE, final residual.

Host-side folding: LN gains into qkv/fc1 weights, LN biases into qkv/fc1
biases, attention 1/sqrt(hd) into W_q, layer-scale gammas into proj/fc2
weights+biases.
"""

import os
import sys

for _p in ("/opt/trn_rl_repo", "/root/.axon_site/_ro/trn_rl_repo"):
    if os.path.isdir(_p) and _p not in sys.path:
        sys.path.insert(0, _p)

import numpy as np
import ml_dtypes
from contextlib import ExitStack

import concourse.bass as bass
import concourse.bacc as bacc
import concourse.mybir as mybir
import concourse.tile as tile
from concourse.alu_op_type import AluOpType
from concourse import bass_utils
from concourse import library_config
from concourse._compat import with_exitstack

AF = mybir.ActivationFunctionType
BF16 = mybir.dt.bfloat16
F32 = mybir.dt.float32
nbf16 = ml_dtypes.bfloat16

# model dims
D = 768
H = 12
HD = 64
F3 = 3 * D
MLP = 4 * D
LN_EPS = 1e-5
P = 128
DC = D // P            # 6 d-chunks
FC = MLP // P          # 24 mlp feature chunks
N_CORES = 8

MUL = AluOpType.mult
ADD = AluOpType.add


def _ln1_chunk(nc, work, tp_ps, x_t, eps, zero, xnT, t, ident):
    """LayerNorm a [128, 768] token tile and write its transpose into xnT."""
    s6 = work.tile([P, 2, 6], F32, tag="s6")
    mv = work.tile([P, 2], F32, tag="mv")
    nc.vector.bn_stats(s6[:, 0, :], x_t[:, 0:384])
    nc.vector.bn_stats(s6[:, 1, :], x_t[:, 384:768])
    nc.vector.bn_aggr(mv[:], s6[:])
    std = work.tile([P, 1], F32, tag="std")
    rstd = work.tile([P, 1], F32, tag="rstd")
    nmr = work.tile([P, 1], F32, tag="nmr")
    nc.scalar.activation(std[:], mv[:, 1:2], AF.Sqrt, bias=eps[:], scale=1.0)
    nc.vector.reciprocal(rstd[:], std[:])
    nc.vector.tensor_scalar(nmr[:], mv[:, 0:1], rstd[:], -1.0, MUL, MUL)
    xn = work.tile([P, D], BF16, tag="xn")
    nc.scalar.activation(xn[:], x_t[:], AF.Identity, bias=nmr[:], scale=rstd[:])
    for c in range(DC):
        tp = tp_ps.tile([P, P], BF16, tag="tp")
        nc.tensor.transpose(tp[:], xn[:, c * P:(c + 1) * P], ident[:])
        nc.vector.tensor_copy(xnT[c][:, t * P:(t + 1) * P], tp[:])


@with_exitstack
def _block_kernel(ctx: ExitStack, tc: tile.TileContext, io, NB, SN, nzbias=frozenset()):
    """io: dict of DRAM APs. NB sequences of length SN on this core."""
    nc = tc.nc
    T = NB * SN                  # tokens on this core
    NT = T // P                  # token tiles
    NKT = SN // P                # k-tiles per sequence
    QCW = min(512, SN)           # q-chunk width
    NQC = SN // QCW              # q-chunks per sequence
    TCW = min(512, T)            # token-chunk width for qk gemm
    NTC = T // TCW
    MQW = min(512, T)            # mlp quarter width
    NMQ = T // MQW

    const = ctx.enter_context(tc.tile_pool(name="const", bufs=1))
    work = ctx.enter_context(tc.tile_pool(name="work", bufs=3))

    # partition_broadcast lives in the 'attn' gpsimd library
    nc.gpsimd.load_library(library_config.attn)

    ident = const.tile([P, P], BF16, tag="ident")
    nc.sync.dma_start(ident[:], io["ident"][:])
    ones512 = const.tile([1, 512], BF16, tag="ones512")
    nc.vector.memset(ones512[:], 1.0)
    ones128 = const.tile([1, P], BF16, tag="ones128")
    nc.vector.memset(ones128[:], 1.0)
    eps = const.tile([P, 1], F32, tag="eps")
    nc.vector.memset(eps[:], LN_EPS)
    zero = const.tile([P, 1], F32, tag="zero")
    nc.vector.memset(zero[:], 0.0)
    bqkv = const.tile([1, F3], BF16, tag="bqkv")
    nc.sync.dma_start(bqkv[:], io["bqkv"][:])
    bproj = const.tile([1, D], BF16, tag="bproj")
    nc.sync.dma_start(bproj[:], io["bproj"][:])
    bfc1 = const.tile([1, MLP], BF16, tag="bfc1")
    nc.sync.dma_start(bfc1[:], io["bfc1"][:])
    bfc2 = const.tile([1, D], BF16, tag="bfc2")
    nc.sync.dma_start(bfc2[:], io["bfc2"][:])

    # attn output, d-major [d, tokens], persistent until proj
    atp = ctx.enter_context(tc.tile_pool(name="attnT", bufs=1))
    attnT = [atp.tile([P, T], BF16, tag=f"at{c}", name=f"at{c}") for c in range(DC)]

    # ---------------- phases 1-3: LN1, QKV, attention ----------------
    with ExitStack() as attn_scope:
        wqp = attn_scope.enter_context(tc.tile_pool(name="wqkv", bufs=1))
        wqkv = [wqp.tile([P, F3], BF16, tag=f"wq{c}", name=f"wq{c}") for c in range(DC)]
        for c in range(DC):
            nc.sync.dma_start(wqkv[c][:], io["wqkv"][c * P:(c + 1) * P, :])

        xnp = attn_scope.enter_context(tc.tile_pool(name="xnT", bufs=1))
        xnT = [xnp.tile([P, T], BF16, tag=f"xnT{c}", name=f"xnT{c}") for c in range(DC)]
        vp = attn_scope.enter_context(tc.tile_pool(name="vaug", bufs=1))
        vaug = [vp.tile([P, H * (HD + 1)], BF16, tag=f"v{g}", name=f"v{g}") for g in range(NT)]
        qkp = attn_scope.enter_context(tc.tile_pool(name="qkT", bufs=2))
        php = attn_scope.enter_context(tc.tile_pool(name="phat", bufs=2 * NKT + 4))

        with ExitStack() as p1_scope:
            xres = p1_scope.enter_context(tc.tile_pool(name="xres", bufs=4))
            tp_ps = p1_scope.enter_context(
                tc.tile_pool(name="tp_ps", bufs=2, space="PSUM"))
            mm_ps = p1_scope.enter_context(
                tc.tile_pool(name="mm_ps", bufs=2, space="PSUM"))

            # LN1 + transpose, per token tile
            for t in range(NT):
                x_t = xres.tile([P, D], F32, tag="x")
                nc.sync.dma_start(x_t[:], io["x"][t * P:(t + 1) * P, :])
                _ln1_chunk(nc, work, tp_ps, x_t, eps, zero, xnT, t, ident)

            # v GEMM: token-major, augmented with ones columns
            for g in range(NT):
                nc.vector.memset(
                    vaug[g][:].rearrange("p (h c) -> p h c", c=HD + 1)[:, :, HD:],
                    1.0)
                for vn in range(2):
                    ps = mm_ps.tile([P, 384], F32, tag="vmm")
                    for c in range(DC):
                        nc.tensor.matmul(
                            ps[:], xnT[c][:, g * P:(g + 1) * P],
                            wqkv[c][:, 2 * D + vn * 384: 2 * D + (vn + 1) * 384],
                            start=(c == 0),
                            stop=(c == DC - 1 and "bqkv" not in nzbias))
                    if "bqkv" in nzbias:
                        nc.tensor.matmul(
                            ps[:], ones128[:],
                            bqkv[:, 2 * D + vn * 384: 2 * D + (vn + 1) * 384],
                            start=False, stop=True)
                    # scatter 6 head-halves into the augmented layout
                    dst = vaug[g][:].rearrange("p (h c) -> p h c", c=HD + 1)
                    nc.vector.tensor_copy(
                        dst[:, vn * 6:(vn + 1) * 6, 0:HD],
                        ps[:].rearrange("p (h c) -> p h c", c=HD))

        # per head-pair: qk GEMM then attention for its 2 heads x NB batches
        with ExitStack() as p3_scope:
            qk_ps = p3_scope.enter_context(
                tc.tile_pool(name="qk_ps", bufs=2, space="PSUM"))
            sc_ps = p3_scope.enter_context(
                tc.tile_pool(name="sc_ps", bufs=2, space="PSUM"))
            pv_ps = p3_scope.enter_context(
                tc.tile_pool(name="pv_ps", bufs=2, space="PSUM"))

            for p in range(H // 2):
                qT = qkp.tile([P, T], BF16, tag="qT")
                kT = qkp.tile([P, T], BF16, tag="kT")
                for dst, base in ((qT, p * P), (kT, D + p * P)):
                    for tn in range(NTC):
                        ps = qk_ps.tile([P, TCW], F32, tag="qkmm")
                        for c in range(DC):
                            nc.tensor.matmul(
                                ps[:], wqkv[c][:, base:base + P],
                                xnT[c][:, tn * TCW:(tn + 1) * TCW],
                                start=(c == 0),
                                stop=(c == DC - 1 and "bqkv" not in nzbias))
                        if "bqkv" in nzbias:
                            nc.tensor.matmul(
                                ps[:], bqkv[:, base:base + P],
                                ones512[:, 0:TCW], start=False, stop=True)
                        nc.vector.tensor_copy(
                            dst[:, tn * TCW:(tn + 1) * TCW], ps[:])

                for b in range(NB):
                    # scores + exp for BOTH heads first, then both PV chains:
                    # keeps PE streaming head h1's scores while ScalarE exps
                    # head h0, instead of stalling PE on the exp latency.
                    phat = {}
                    for h2 in range(2):
                        hb = h2 * HD
                        phat[h2] = []
                        for kt in range(NKT):
                            sc = sc_ps.tile([P, SN], F32, tag="sc")
                            for qc in range(NQC):
                                # single matmul per bank region: start=True
                                # (a 2-bank tile only clears the bank each
                                # matmul actually targets)
                                nc.tensor.matmul(
                                    sc[:, qc * QCW:(qc + 1) * QCW],
                                    kT[hb:hb + HD,
                                       b * SN + kt * P: b * SN + (kt + 1) * P],
                                    qT[hb:hb + HD,
                                       b * SN + qc * QCW: b * SN + (qc + 1) * QCW],
                                    start=True, stop=True)
                            ph = php.tile([P, SN], BF16, tag="ph")
                            nc.scalar.activation(ph[:], sc[:], AF.Exp,
                                                 bias=zero[:], scale=1.0)
                            phat[h2].append(ph)
                    for h2 in range(2):
                        hb = h2 * HD
                        h = 2 * p + h2
                        for qc in range(NQC):
                            pv = pv_ps.tile([HD + 1, QCW], F32, tag="pv")
                            for kt in range(NKT):
                                nc.tensor.matmul(
                                    pv[:],
                                    vaug[b * NKT + kt][:,
                                        h * (HD + 1): (h + 1) * (HD + 1)],
                                    phat[h2][kt][:, qc * QCW:(qc + 1) * QCW],
                                    start=(kt == 0), stop=(kt == NKT - 1),
                                    skip_group_check=True)
                            inv = work.tile([1, QCW], F32, tag="inv")
                            nc.vector.reciprocal(inv[:], pv[HD:HD + 1, :])
                            invb = work.tile([HD, QCW], F32, tag="invb")
                            nc.gpsimd.partition_broadcast(invb[:], inv[:])
                            nc.vector.tensor_tensor(
                                attnT[p][hb:hb + HD,
                                         b * SN + qc * QCW: b * SN + (qc + 1) * QCW],
                                pv[0:HD, :], invb[:], MUL)

    # ---------------- phases 4-5: proj+LN2, MLP ----------------
    with ExitStack() as tail_scope:
        yp = tail_scope.enter_context(tc.tile_pool(name="y", bufs=1))
        ytiles = [yp.tile([P, D], F32, tag=f"y{t}", name=f"y{t}") for t in range(NT)]
        ynp = tail_scope.enter_context(tc.tile_pool(name="ynT", bufs=1))
        ynT = [ynp.tile([P, T], BF16, tag=f"ynT{c}", name=f"ynT{c}") for c in range(DC)]

        with ExitStack() as p4_scope:
            wpp = p4_scope.enter_context(tc.tile_pool(name="wproj", bufs=1))
            wproj = [wpp.tile([P, D], BF16, tag=f"wp{c}", name=f"wp{c}") for c in range(DC)]
            for c in range(DC):
                nc.sync.dma_start(wproj[c][:], io["wproj"][c * P:(c + 1) * P, :])
            xres = p4_scope.enter_context(tc.tile_pool(name="xres2", bufs=4))
            tp_ps = p4_scope.enter_context(
                tc.tile_pool(name="tp_ps2", bufs=2, space="PSUM"))
            pr_ps = p4_scope.enter_context(
                tc.tile_pool(name="pr_ps", bufs=4, space="PSUM"))

            for t in range(NT):
                x_t = xres.tile([P, D], F32, tag="x2")
                nc.sync.dma_start(x_t[:], io["x"][t * P:(t + 1) * P, :])
                y_t = ytiles[t]
                for nn in range(2):
                    pr = pr_ps.tile([P, 384], F32, tag="pr")
                    for c in range(DC):
                        nc.tensor.matmul(
                            pr[:], attnT[c][:, t * P:(t + 1) * P],
                            wproj[c][:, nn * 384:(nn + 1) * 384],
                            start=(c == 0),
                            stop=(c == DC - 1 and "bproj" not in nzbias))
                    if "bproj" in nzbias:
                        nc.tensor.matmul(pr[:], ones128[:],
                                         bproj[:, nn * 384:(nn + 1) * 384],
                                         start=False, stop=True)
                    nc.vector.tensor_tensor(
                        y_t[:, nn * 384:(nn + 1) * 384], pr[:],
                        x_t[:, nn * 384:(nn + 1) * 384], ADD)
                _ln1_chunk(nc, work, tp_ps, y_t, eps, zero, ynT, t, ident)

        with ExitStack() as p5_scope:
            w1p = p5_scope.enter_context(tc.tile_pool(name="wfc1", bufs=4))
            w2p = p5_scope.enter_context(tc.tile_pool(name="wfc2", bufs=1))
            wfc2 = [w2p.tile([P, D], BF16, tag=f"w2{f}", name=f"w2{f}") for f in range(FC)]
            for f in range(FC):
                nc.sync.dma_start(wfc2[f][:], io["wfc2"][f * P:(f + 1) * P, :])
            glp = p5_scope.enter_context(tc.tile_pool(name="gelu", bufs=1))
            m1_ps = p5_scope.enter_context(
                tc.tile_pool(name="m1_ps", bufs=3, space="PSUM"))
            m2_ps = p5_scope.enter_context(
                tc.tile_pool(name="m2_ps", bufs=4, space="PSUM"))

            for qq in range(NMQ):
                gelu = []
                for f in range(FC):
                    wf1 = w1p.tile([P, D], BF16, tag="wf1")
                    nc.sync.dma_start(
                        wf1[:].rearrange("p (c j) -> p c j", j=P),
                        io["wfc1"][f].rearrange("c p j -> p c j"))
                    m1 = m1_ps.tile([P, MQW], F32, tag="m1")
                    for c in range(DC):
                        nc.tensor.matmul(
                            m1[:], wf1[:, c * P:(c + 1) * P],
                            ynT[c][:, qq * MQW:(qq + 1) * MQW],
                            start=(c == 0),
                            stop=(c == DC - 1 and "bfc1" not in nzbias))
                    if "bfc1" in nzbias:
                        nc.tensor.matmul(m1[:], bfc1[:, f * P:(f + 1) * P],
                                         ones512[:, 0:MQW],
                                         start=False, stop=True)
                    gl = glp.tile([P, MQW], BF16, tag=f"gl{f}", name=f"gl{f}")
                    nc.scalar.activation(gl[:], m1[:], AF.Gelu,
                                         bias=zero[:], scale=1.0)
                    gelu.append(gl)
                for tt in range(MQW // P):
                    t = qq * (MQW // P) + tt
                    y_t = ytiles[t]
                    for nn in range(2):
                        m2 = m2_ps.tile([P, 384], F32, tag="m2")
                        for f in range(FC):
                            nc.tensor.matmul(
                                m2[:], gelu[f][:, tt * P:(tt + 1) * P],
                                wfc2[f][:, nn * 384:(nn + 1) * 384],
                                start=(f == 0),
                                stop=(f == FC - 1 and "bfc2" not in nzbias))
                        if "bfc2" in nzbias:
                            nc.tensor.matmul(m2[:], ones128[:],
                                             bfc2[:, nn * 384:(nn + 1) * 384],
                                             start=False, stop=True)
                        nc.vector.tensor_tensor(
                            y_t[:, nn * 384:(nn + 1) * 384], m2[:],
                            y_t[:, nn * 384:(nn + 1) * 384], ADD)
                    nc.sync.dma_start(io["z"][t * P:(t + 1) * P, :], y_t[:])


def build_program(NB=2, SN=1024, enable_asserts=False, nzbias=frozenset(("bqkv", "bproj", "bfc1", "bfc2"))):
    T = NB * SN
    nc = bacc.Bacc("TRN2", target_bir_lowering=False, debug=False,
                   enable_asserts=enable_asserts)
    io = {
        "x": nc.dram_tensor("x", [T, D], F32, kind="ExternalInput").ap(),
        "wqkv": nc.dram_tensor("wqkv", [D, F3], BF16, kind="ExternalInput").ap(),
        "bqkv": nc.dram_tensor("bqkv", [1, F3], BF16, kind="ExternalInput").ap(),
        "wproj": nc.dram_tensor("wproj", [D, D], BF16, kind="ExternalInput").ap(),
        "bproj": nc.dram_tensor("bproj", [1, D], BF16, kind="ExternalInput").ap(),
        "wfc1": nc.dram_tensor("wfc1", [FC, DC, P, P], BF16, kind="ExternalInput").ap(),
        "bfc1": nc.dram_tensor("bfc1", [1, MLP], BF16, kind="ExternalInput").ap(),
        "wfc2": nc.dram_tensor("wfc2", [MLP, D], BF16, kind="ExternalInput").ap(),
        "bfc2": nc.dram_tensor("bfc2", [1, D], BF16, kind="ExternalInput").ap(),
        "ident": nc.dram_tensor("ident", [P, P], BF16, kind="ExternalInput").ap(),
        "z": nc.dram_tensor("z", [T, D], F32, kind="ExternalOutput").ap(),
    }
    with tile.TileContext(nc) as tc:
        _block_kernel(tc, io, NB, SN, nzbias)
    nc.compile()
    return nc


def fold_weights(ln1_g, ln1_b, qkv_w, qkv_b, proj_w, proj_b, gamma1,
                 ln2_g, ln2_b, fc1_w, fc1_b, fc2_w, fc2_b, gamma2):
    """Host-side folding; returns bf16 weight dict shared by all cores."""
    f32 = np.float32
    qkv_w = np.asarray(qkv_w, f32)
    w_qkv = np.asarray(ln1_g, f32)[:, None] * qkv_w
    b_qkv = np.asarray(qkv_b, f32) + np.asarray(ln1_b, f32) @ qkv_w
    scale = (D // H) ** -0.5
    w_qkv[:, 0:D] *= scale
    b_qkv[0:D] *= scale
    w_proj = np.asarray(proj_w, f32) * np.asarray(gamma1, f32)[None, :]
    b_proj = np.asarray(proj_b, f32) * np.asarray(gamma1, f32)
    fc1_w = np.asarray(fc1_w, f32)
    w_fc1 = np.asarray(ln2_g, f32)[:, None] * fc1_w
    b_fc1 = np.asarray(fc1_b, f32) + np.asarray(ln2_b, f32) @ fc1_w
    w_fc2 = np.asarray(fc2_w, f32) * np.asarray(gamma2, f32)[None, :]
    b_fc2 = np.asarray(fc2_b, f32) * np.asarray(gamma2, f32)
    return {
        "wqkv": w_qkv.astype(nbf16), "bqkv": b_qkv[None, :].astype(nbf16),
        "wproj": w_proj.astype(nbf16), "bproj": b_proj[None, :].astype(nbf16),
        "wfc1": np.ascontiguousarray(
            w_fc1.reshape(DC, P, FC, P).transpose(2, 0, 1, 3)).astype(nbf16),
        "bfc1": b_fc1[None, :].astype(nbf16),
        "wfc2": w_fc2.astype(nbf16), "bfc2": b_fc2[None, :].astype(nbf16),
        "ident": np.eye(P, dtype=np.float32).astype(nbf16),
    }


_PROGRAM_CACHE = {}


def _get_program(NB, SN, nzbias):
    key = (NB, SN, nzbias)
    if key not in _PROGRAM_CACHE:
        _PROGRAM_CACHE[key] = build_program(NB, SN, nzbias=nzbias)
    return _PROGRAM_CACHE[key]


def kernel(x, ln1_g, ln1_b, qkv_w, qkv_b, proj_w, proj_b, gamma1,
           ln2_g, ln2_b, fc1_w, fc1_b, fc2_w, fc2_b, gamma2,
           _trace=False, _trace_kwargs=None):
    x = np.asarray(x, np.float32)
    B, N, _ = x.shape
    NB = B // N_CORES
    shared = fold_weights(ln1_g, ln1_b, qkv_w, qkv_b, proj_w, proj_b, gamma1,
                          ln2_g, ln2_b, fc1_w, fc1_b, fc2_w, fc2_b, gamma2)
    nzbias = frozenset(n for n in ("bqkv", "bproj", "bfc1", "bfc2")
                       if np.any(shared[n].astype(np.float32) != 0.0))
    nc = _get_program(NB, N, nzbias)
    in_maps = []
    for c in range(N_CORES):
        m = dict(shared)
        m["x"] = np.ascontiguousarray(
            x[c * NB:(c + 1) * NB].reshape(NB * N, D))
        in_maps.append(m)
    res = bass_utils.run_bass_kernel_spmd(
        nc, in_maps, core_ids=list(range(N_CORES)),
        trace=_trace, **(_trace_kwargs or {}))
    out = np.stack([res.results[c]["z"].reshape(NB, N, D)
                    for c in range(N_CORES)])
    out = out.reshape(B, N, D)
    if _trace:
        kernel._last_results = res
    return out
